# revision 42
# baseline (speedup 1.0000x reference)
"""Trainium2 Bass kernel for the BaseMemory coref scoring module.

Computes, for full inputs (M=65536 memory slots, D=768, E=20, H=64):
    score = relu(pair @ W1 + b1) @ W2 + b2, masked with ent_counter>0,
    where pair = [mem, ment, mem*ment, dist_emb, cnt_emb].

Sharding: data-parallel over the cluster dimension M across 8 NeuronCores.

The kernel is memory-bound on streaming mem_vectors, so everything that can
be folded away is folded away on the host (exact f64 algebra, O(M*D) data
prep only — all O(M*D*H) FLOPs stay on device):

  - mem@W1_mem + (mem*ment)@W1_had = mem @ (W1_mem + diag(ment)@W1_had) =: mem @ W
  - the bucketized feature-table + bias contribution t[m] (a 64-vector from a
    100-entry table) is folded INTO the mem vectors: with A = W (W^T W)^-1,
    W^T (mem + A t) = W^T mem + t exactly.  So the device computes just
        score = W2^T relu(W^T x')   with x' = mem + A t[idx]
  - bucket indices are computed integer-exactly on host (frexp, no float log)
  - the ent_counter<=0 mask (-10000) and the +b2 offset are applied on host
    after gathering the per-core outputs.

x' streams as bf16 (12.6 MiB/core instead of 24 MiB f32), halving the HBM
traffic that bounds the kernel; scores accumulate in fp32 PSUM, keeping the
max relative error ~1e-3 (tolerance 2e-2).
"""

import os
import numpy as np

# The bass kernel executes through the axon PJRT backend; make sure jax can
# see it even if the caller pinned JAX_PLATFORMS (e.g. to "cpu").
_jp = os.environ.get("JAX_PLATFORMS")
if _jp is not None and _jp != "" and "axon" not in _jp:
    os.environ["JAX_PLATFORMS"] = "axon," + _jp

M, D, E, H = 65536, 768, 20, 64
N_CORES = 8
MS = M // N_CORES          # rows per core = 8192
GROUP = 512                # rows per PE matmul group
NG = MS // GROUP           # 16 groups per core
KCH = D // 128             # 6 contraction chunks
PAIR_DMA = 2               # groups per x DMA

_CACHE = {}


def _build():
    """Build + compile the 8-core SPMD bass program once per process."""
    if "nc" in _CACHE:
        return _CACHE["nc"]

    import concourse.bass as bass
    import concourse.mybir as mybir
    import concourse.tile as tile
    from concourse import bacc

    F32 = mybir.dt.float32
    F32R = mybir.dt.float32r
    BF16 = mybir.dt.bfloat16

    nc = bacc.Bacc("TRN2", target_bir_lowering=False, debug=False,
                   enable_asserts=False, num_devices=N_CORES)

    # Single input tensor: [w1 table (385 cols) | 16 groups x 3072 cols].
    # The weights ride at the head of the xq stream on the SAME queue, so
    # they land with the first chunk (~12us) instead of trickling in tiny
    # descriptors on a second queue that the stream starves (~18us).
    WCOLS = KCH * H + 1
    GCOLS = KCH * GROUP
    xq_d = nc.dram_tensor("xq", [128, WCOLS + NG * GCOLS], BF16,
                          kind="ExternalInput").ap()
    out_d = nc.dram_tensor("out", [MS], F32, kind="ExternalOutput").ap()

    # host layout: xq[p, WCOLS + g*GCOLS + k*GROUP + c] = x'[128k+p, 512g+c]
    out_r = out_d.rearrange("(q c) -> q c", q=NG // 4)  # [4, 2048]
    out_g = out_d.rearrange("(g c) -> g c", g=NG)       # [16, 512]

    relu = mybir.ActivationFunctionType.Relu

    with tile.TileContext(nc) as tc:
        with (
            tc.tile_pool(name="consts", bufs=1) as cpool,
            tc.tile_pool(name="xin", bufs=8) as px,
            tc.tile_pool(name="ht", bufs=4) as pht,
            tc.tile_pool(name="osb", bufs=4) as posb,
            tc.tile_pool(name="psz", bufs=4, space="PSUM") as psz,
            tc.tile_pool(name="pss", bufs=3, space="PSUM") as pss,
        ):
            # first DMA on the sync queue: the weight table, persistent
            cw = cpool.tile([128, WCOLS], BF16, tag="cw")
            nc.sync.dma_start(cw[:], xq_d[:, 0:WCOLS])
            w1t = cw[:, 0:WCOLS]
            wsc = cw[0:H, KCH * H:WCOLS]                # W2 column, [64, 1]

            # remaining groups stream as single-group DMAs: finer-grained
            # completion semaphores hide the ~1us receipt latency that
            # otherwise stalls the PE ~0.5us at every chunk boundary
            def load_group(g):
                xk = px.tile([128, GCOLS], BF16, tag="xin")
                if g == 0:
                    # per-k-chunk loads so the first matmul starts as soon
                    # as the first 128KB lands instead of the full 768KB
                    for k in range(KCH):
                        nc.sync.dma_start(
                            xk[:, k * GROUP:(k + 1) * GROUP],
                            xq_d[:, WCOLS + k * GROUP:
                                 WCOLS + (k + 1) * GROUP])
                else:
                    nc.sync.dma_start(xk[:],
                                      xq_d[:, WCOLS + g * GCOLS:
                                           WCOLS + (g + 1) * GCOLS])
                return xk

            PREFETCH = 6
            tiles = {g: load_group(g) for g in range(PREFETCH)}
            osb_tiles = {}
            pending = None

            def emit_score(gp, htp, last=False):
                sc = pss.tile([1, GROUP], F32, tag="pss")
                if last:
                    # two half matmuls so each can fire as soon as its
                    # relu half (split across Scalar/Vector) completes
                    half = GROUP // 2
                    nc.tensor.matmul(sc[0:1, 0:half], wsc[:],
                                     htp[:, 0:half], start=True, stop=True)
                    nc.tensor.matmul(sc[0:1, half:], wsc[:],
                                     htp[:, half:], start=True, stop=True)
                else:
                    nc.tensor.matmul(sc[:], wsc[:], htp[:], start=True,
                                     stop=True)
                if gp < 12:
                    q = gp // 4
                    if gp % 4 == 0:
                        osb_t = posb.tile([1, 4 * GROUP], F32, tag="osb")
                        osb_tiles[q] = osb_t
                    orow = osb_tiles[q][0:1, GROUP * (gp % 4):
                                        GROUP * (gp % 4 + 1)]
                    nc.vector.tensor_copy(orow, sc[:])
                    if gp % 4 == 3:
                        nc.sync.dma_start(out_r[q:q + 1, :],
                                          osb_tiles.pop(q)[:])
                else:
                    # last four groups: per-group 2KB stores; final copy is
                    # split across both engines to shorten the tail chain
                    osb_t = posb.tile([1, GROUP], F32, tag="osbg")
                    if last:
                        half = GROUP // 2
                        nc.vector.tensor_copy(osb_t[0:1, 0:half],
                                              sc[0:1, 0:half])
                        nc.scalar.copy(osb_t[0:1, half:], sc[0:1, half:])
                    else:
                        nc.vector.tensor_copy(osb_t[:], sc[:])
                    # sync queue is idle here (all loads already issued) and
                    # its descriptor-gen doesn't occupy the Scalar engine
                    nc.sync.dma_start(out_g[gp:gp + 1, :], osb_t[:])

            for g in range(NG):
                t = g + PREFETCH
                if t < NG and t not in tiles:
                    tiles[t] = load_group(t)
                xg = tiles[g][:]

                zt = psz.tile([H, GROUP], F32, tag="psz")
                for k in range(KCH):
                    nc.tensor.matmul(zt[:], w1t[:, k * H:(k + 1) * H],
                                     xg[:, k * GROUP:(k + 1) * GROUP],
                                     start=(k == 0), stop=(k == KCH - 1))

                # relu first so it leads this iteration's Scalar/Vector
                # FIFO; the previous group's score/copy/store work follows.
                ht = pht.tile([H, GROUP], BF16, tag="ht")
                if g == NG - 1:
                    # split the final relu across both engines to shorten
                    # the tail chain
                    half = GROUP // 2
                    nc.vector.tensor_scalar_max(ht[:, 0:half],
                                                zt[:, 0:half], 0.0)
                    nc.scalar.activation(ht[:, half:], zt[:, half:], relu)
                elif g % 2 == 0:
                    nc.scalar.activation(ht[:], zt[:], relu)
                else:
                    nc.vector.tensor_scalar_max(ht[:], zt[:], 0.0)

                # score matmul for the previous group goes to the PE AFTER
                # this group's z matmuls, so its relu has time to finish and
                # the PE never stalls on the Scalar engine.
                if pending is not None:
                    emit_score(*pending)
                pending = (g, ht)

            emit_score(pending[0], pending[1], last=True)

    nc.compile()
    _CACHE["nc"] = nc
    return nc


def _bucket(c):
    """Integer-exact replica of the reference's get_bucket (identity <=4,
    floor(log2)+3 above, clipped to [0, 9])."""
    c = np.asarray(c, np.int64)
    cpos = np.maximum(c, 1).astype(np.float64)
    lg = np.frexp(cpos)[1] - 1          # exact floor(log2) for integers
    idx = np.where(c <= 4, c, lg + 3)
    return np.clip(idx, 0, 9).astype(np.int64)


def _prepare_maps(ment_emb, mem_vectors, dist_table, counter_table,
                  W1, b1, W2, b2, ent_counter, last_mention_start,
                  ment_start):
    import ml_dtypes

    f64 = np.float64
    ment = np.asarray(ment_emb, f64)
    W1 = np.asarray(W1, f64)
    W1m, W1r, W1h = W1[0:D], W1[D:2 * D], W1[2 * D:3 * D]
    W1d, W1c = W1[3 * D:3 * D + E], W1[3 * D + E:3 * D + 2 * E]

    w1eff = W1m + ment[:, None] * W1h                       # [768, 64]
    bias_vec = np.asarray(b1, f64) + ment @ W1r             # [64]
    T_d = np.asarray(dist_table, f64) @ W1d                 # [10, 64]
    T_c = np.asarray(counter_table, f64) @ W1c              # [10, 64]
    # t(bd, bc) = T_d[bd] + T_c[bc] + bias_vec, for all 100 bucket combos
    T_comb = (T_d[:, None, :] + T_c[None, :, :] +
              bias_vec).reshape(100, H)                     # [100, 64]

    # delta[r] solves w1eff^T delta = T_comb[r] (min-norm): the feature/bias
    # contribution is folded into the mem vectors themselves.
    G = w1eff.T @ w1eff
    G += np.eye(H) * (1e-12 * np.trace(G) / H)              # ridge, paranoia
    delta_table = (np.linalg.solve(G, T_comb.T).T @ w1eff.T)  # [100, 768]

    cnt = np.asarray(ent_counter, np.int64)
    dist = int(np.asarray(ment_start)) - np.asarray(last_mention_start,
                                                    np.int64)
    idx = _bucket(dist) * 10 + _bucket(cnt)                 # [M]

    xprime = np.asarray(mem_vectors, np.float32)
    xprime = xprime + delta_table.astype(np.float32)[idx]   # [M, 768]

    # [p, (k n)] swizzle (d = 128k + p) + last column = W2; the weight
    # table rides at the head of each core's xq stream (one DRAM tensor)
    w1b = np.zeros((128, KCH * H + 1), np.float64)
    w1b[:, :KCH * H] = w1eff.reshape(KCH, 128, H).transpose(1, 0, 2) \
                            .reshape(128, KCH * H)
    w1b[:H, KCH * H] = np.asarray(W2, np.float64).reshape(-1)
    w1b = w1b.astype(ml_dtypes.bfloat16)

    in_maps = []
    for c in range(N_CORES):
        xc = xprime[c * MS:(c + 1) * MS]                    # [8192, 768]
        # -> [p, g, k, c] with d = 128k+p, m = 512g+c
        xt = xc.T.reshape(KCH, 128, NG, GROUP).transpose(1, 2, 0, 3)
        xq = np.ascontiguousarray(xt).astype(ml_dtypes.bfloat16)
        in_maps.append(dict(xq=np.concatenate(
            [w1b, xq.reshape(128, NG * KCH * GROUP)], axis=1)))

    b2v = float(np.asarray(b2, np.float64).reshape(-1)[0])
    return in_maps, (cnt <= 0), b2v


def _postprocess(results, masked, b2v):
    out = np.empty(M + 1, np.float32)
    for c in range(N_CORES):
        out[c * MS:(c + 1) * MS] = results[c]["out"]
    if b2v != 0.0:
        out[:M] += np.float32(b2v)
    out[:M][masked] = -10000.0
    out[M] = 0.0
    return out


def run_spmd(in_maps, trace=False):
    from concourse.bass_utils import run_bass_kernel_spmd
    nc = _build()
    return run_bass_kernel_spmd(nc, in_maps, list(range(N_CORES)),
                                trace=trace)


def kernel(**inputs):
    in_maps, masked, b2v = _prepare_maps(**inputs)
    res = run_spmd(in_maps, trace=False)
    return _postprocess(res.results, masked, b2v)
